# revision 1
# baseline (speedup 1.0000x reference)
"""Llama GQA causal attention layer (the "topk" in the module name is a
mathematical identity) on 8 Trainium2 NeuronCores.

Sharding: tensor-parallel over heads. Each core owns 2 of the 16 q-heads and
the single kv-head they share, computes its slice of Q/K/V projections, RoPE,
causal flash attention (scores kept on-chip in [k, q] orientation), and a
row-slice o_proj producing a full-shape [S, HID] partial; the host sums the 8
partials (the o_proj row-parallel AllReduce done on host).

Shapes hardcoded per problem spec:
  hidden_states [1, 4096, 2048] f32, position_ids [1, 4096] i32,
  Wq [2048, 2048], Wk/Wv [2048, 512], Wo [2048, 2048] f32.
"""

import math
import os
import sys

import numpy as np

if "/opt/trn_rl_repo" not in sys.path:
    sys.path.insert(0, "/opt/trn_rl_repo")

import concourse.bass as bass
import concourse.mybir as mybir
import concourse.tile as tile
from concourse import bacc, bass_utils

B, S, HID = 1, 4096, 2048
NH, KVH, HD = 16, 4, 128
GROUPS = NH // KVH
NCORES = 8
HPC = NH // NCORES          # q heads per core = 2
ST = S // 128               # 32 s-tiles
KT = HID // 128             # 16 hid-tiles (contraction)
QCH = 512                   # q chunk width for attention
NQC = S // QCH
ROPE_THETA = 10000.0
ISQ = 1.0 / math.sqrt(HD)

F32 = mybir.dt.float32
F32R = mybir.dt.float32r

# matmul operand dtype: float32r streams at 1 cyc/row (vs 4 for float32) when
# the moving dim >= 256; storage bytes are identical to f32 so we bitcast APs
# at the matmul callsites only.
USE_F32R = os.environ.get("KERNEL_MM_DT", "f32r") == "f32r"


def _r(ap):
    return ap


def build_body(tc, out, ht, wqkv, wo, cosr, sinr, identd, trimaskd, onesd):
    """Emit the per-core program, software-pipelined per 512-token q-chunk:
    for each chunk: [A] project+RoPE+transpose 4 s-tiles -> [B] causal flash
    attention for both heads -> [C] o_proj rows of this chunk. Emission order
    sets Tile priorities so PE stays dense across stage boundaries.

    DRAM layouts (host pre-arranged, partition dim first):
      ht    [128, KT, S]   ht[p, t, s]  = hidden[s, 128 t + p]
      wqkv  [128, KT, 512] wqkv[p,t,j]  = [Wq_c | Wk_c | Wv_c][128 t + p, j]
      wo    [128, HPC, HID] wo[p, j, n] = Wo[256 c + 128 j + p, n]
      cosr/sinr [128, ST, 64] cosr[p, st, f] = cos[128 st + p, f]
      out   [S, HID] partial output (sum over cores on host)
    """
    nc = tc.nc

    with (
        tc.tile_pool(name="const", bufs=1) as constp,
        tc.tile_pool(name="slabs", bufs=1) as slabs,
        tc.tile_pool(name="stream", bufs=2) as sp,
        tc.tile_pool(name="qtp", bufs=2) as qtp,
        tc.tile_pool(name="atp", bufs=2) as atp,
        tc.tile_pool(name="bsb", bufs=4) as bp,
        tc.tile_pool(name="csb", bufs=2) as cp,
        tc.tile_pool(name="ps512", bufs=4, space="PSUM") as ps512,
        tc.tile_pool(name="attps", bufs=2, space="PSUM") as attp,
        tc.tile_pool(name="denps", bufs=1, space="PSUM") as denp,
        tc.tile_pool(name="ptps", bufs=1, space="PSUM") as ptp,
    ):
        # Resident constants. wqkv split per hid-tile so compute starts early.
        wqkv_sb = constp.tile([128, KT, 512], F32R)
        for t in range(KT):
            nc.sync.dma_start(out=wqkv_sb[:, t, :], in_=wqkv[:, t, :])
        cos_sb = constp.tile([128, ST, 64], F32)
        nc.sync.dma_start(out=cos_sb, in_=cosr)
        sin_sb = constp.tile([128, ST, 64], F32)
        nc.sync.dma_start(out=sin_sb, in_=sinr)
        wo_sb = constp.tile([128, HPC, HID], F32R)
        nc.sync.dma_start(out=wo_sb, in_=wo)
        ident = constp.tile([128, 128], F32R)
        nc.sync.dma_start(out=ident, in_=identd)
        # trimask[ki, qi] = 1.0 where qi >= ki else 0 (valid causal, k-major)
        trimask = constp.tile([128, 128], F32R)
        nc.sync.dma_start(out=trimask, in_=trimaskd)
        ones_col = constp.tile([128, 1], F32R)
        nc.sync.dma_start(out=ones_col, in_=onesd[:, 0:1])

        kTc, vc = {}, {}
        for qc in range(NQC):
            # ---------------- A: project + RoPE + transpose ----------------
            qt = qtp.tile([128, HPC, QCH], F32R, tag="qt")
            kt_c = slabs.tile([128, QCH], F32R, tag=f"kT{qc}")
            v_c = slabs.tile([128, 4, 128], F32R, tag=f"v{qc}")
            kTc[qc], vc[qc] = kt_c, v_c
            for si in range(4):
                st = 4 * qc + si
                s0 = st * 128
                sl = si * 128
                hs = sp.tile([128, KT, 128], F32R, tag="hs")
                nc.sync.dma_start(out=hs, in_=ht[:, :, s0 : s0 + 128])
                pq = ps512.tile([128, 512], F32, tag="ps512")
                for t in range(KT):
                    nc.tensor.matmul(
                        pq,
                        lhsT=hs[:, t, :],
                        rhs=wqkv_sb[:, t, :],
                        start=(t == 0),
                        stop=(t == KT - 1),
                    )
                pqv = pq.rearrange("p (j d) -> p j d", d=128)
                c = cos_sb[:, st, :]
                s_ = sin_sb[:, st, :]
                rot = sp.tile([128, 3, 128], F32R, tag="rot")
                t1 = sp.tile([128, 3, 64], F32, tag="t1")
                t2 = sp.tile([128, 3, 64], F32, tag="t2")
                for j in range(3):
                    a = pqv[:, j, 0:64]
                    b = pqv[:, j, 64:128]
                    nc.vector.tensor_mul(t1[:, j, :], a, c)
                    nc.vector.tensor_mul(t2[:, j, :], b, s_)
                    nc.vector.tensor_sub(rot[:, j, 0:64], t1[:, j, :], t2[:, j, :])
                    nc.vector.tensor_mul(t1[:, j, :], b, c)
                    nc.vector.tensor_mul(t2[:, j, :], a, s_)
                    nc.vector.tensor_add(rot[:, j, 64:128], t1[:, j, :], t2[:, j, :])
                nc.scalar.copy(v_c[:, si, :], pqv[:, 3, :])
                for j, dest, eng in (
                    (0, qt[:, 0, sl : sl + 128], "act"),
                    (1, qt[:, 1, sl : sl + 128], "dve"),
                    (2, kt_c[:, sl : sl + 128], "dve"),
                ):
                    pt = ptp.tile([128, 128], F32R, tag="pt")
                    nc.tensor.transpose(pt, rot[:, j, :], ident)
                    if eng == "act":
                        nc.scalar.copy(dest, pt)
                    else:
                        nc.vector.tensor_copy(dest, pt)

            # ---------------- B: causal flash attention --------------------
            q0 = qc * QCH
            at_c = atp.tile([128, HPC, QCH], F32R, tag="att")
            nkt = 4 * qc + 4
            for h in range(HPC):
                att_ps = attp.tile([128, QCH], F32, tag="attps")
                den_ps = denp.tile([1, QCH], F32, tag="denps")
                for kt in range(nkt):
                    k0 = kt * 128
                    kch, kx = kt // 4, kt % 4
                    off = max(0, k0 - q0)
                    ps = ps512.tile([128, QCH], F32, tag="ps512")
                    nc.tensor.matmul(
                        ps[:, off:],
                        lhsT=kTc[kch][:, 128 * kx : 128 * kx + 128],
                        rhs=qt[:, h, off:QCH],
                        start=True,
                        stop=True,
                    )
                    pt_sb = bp.tile([128, QCH], F32R, tag="pT")
                    nc.scalar.activation(
                        pt_sb[:, off:],
                        ps[:, off:],
                        mybir.ActivationFunctionType.Exp,
                        scale=ISQ,
                    )
                    if kt >= 4 * qc:  # diagonal block: zero strictly-future q
                        nc.vector.tensor_mul(
                            pt_sb[:, off : off + 128],
                            pt_sb[:, off : off + 128],
                            trimask,
                        )
                    nc.tensor.matmul(
                        att_ps[:, off:],
                        lhsT=vc[kch][:, kx, :],
                        rhs=pt_sb[:, off:],
                        start=(kt == 0),
                        stop=(kt == nkt - 1),
                    )
                    nc.tensor.matmul(
                        den_ps[:, off:],
                        lhsT=ones_col,
                        rhs=pt_sb[:, off:],
                        start=(kt == 0),
                        stop=(kt == nkt - 1),
                    )
                den_sb = bp.tile([1, QCH], F32, tag="den")
                nc.vector.tensor_copy(den_sb, den_ps)
                rden = bp.tile([1, QCH], F32, tag="rden")
                nc.vector.reciprocal_approx_fast(rden, den_sb)
                rdb = bp.tile([128, QCH], F32, tag="rdb")
                nc.gpsimd.partition_broadcast(rdb, rden)
                nc.vector.tensor_mul(at_c[:, h, :], att_ps, rdb)

            # ---------------- C: o_proj rows of this chunk -----------------
            for si in range(4):
                st = 4 * qc + si
                s0 = st * 128
                sl = si * 128
                osb = cp.tile([128, HID], F32, tag="osb")
                for nch in range(HID // 512):
                    n0 = nch * 512
                    po = ps512.tile([128, 512], F32, tag="ps512")
                    for j in range(HPC):
                        nc.tensor.matmul(
                            po,
                            lhsT=at_c[:, j, sl : sl + 128],
                            rhs=wo_sb[:, j, n0 : n0 + 512],
                            start=(j == 0),
                            stop=(j == HPC - 1),
                        )
                    if nch % 2 == 0:
                        nc.scalar.copy(osb[:, n0 : n0 + 512], po)
                    else:
                        nc.vector.tensor_copy(osb[:, n0 : n0 + 512], po)
                nc.sync.dma_start(out=out[s0 : s0 + 128, :], in_=osb)


_NC_CACHE = {}


def get_nc():
    key = "nc"
    if key not in _NC_CACHE:
        nc = bacc.Bacc(
            "TRN2",
            debug=False,
            enable_asserts=False,
            target_bir_lowering=False,
        )
        ht = nc.dram_tensor("ht", [128, KT, S], F32R, kind="ExternalInput").ap()
        wqkv = nc.dram_tensor("wqkv", [128, KT, 512], F32R, kind="ExternalInput").ap()
        wo = nc.dram_tensor("wo", [128, HPC, HID], F32R, kind="ExternalInput").ap()
        cosr = nc.dram_tensor("cosr", [128, ST, 64], F32, kind="ExternalInput").ap()
        sinr = nc.dram_tensor("sinr", [128, ST, 64], F32, kind="ExternalInput").ap()
        identd = nc.dram_tensor("identd", [128, 128], F32R, kind="ExternalInput").ap()
        trimaskd = nc.dram_tensor("trimaskd", [128, 128], F32R, kind="ExternalInput").ap()
        onesd = nc.dram_tensor("onesd", [128, 128], F32R, kind="ExternalInput").ap()
        out = nc.dram_tensor("out", [S, HID], F32, kind="ExternalOutput").ap()
        with tile.TileContext(nc) as tc:
            build_body(tc, out, ht, wqkv, wo, cosr, sinr, identd, trimaskd, onesd)
        nc.compile()
        _NC_CACHE[key] = nc
    return _NC_CACHE[key]


def prep_in_maps(hidden_states, position_ids, Wq, Wk, Wv, Wo):
    hid = np.asarray(hidden_states, dtype=np.float32)[0]          # [S, HID]
    pos = np.asarray(position_ids)[0].astype(np.float32)          # [S]
    Wq = np.asarray(Wq, dtype=np.float32)
    Wk = np.asarray(Wk, dtype=np.float32)
    Wv = np.asarray(Wv, dtype=np.float32)
    Wo = np.asarray(Wo, dtype=np.float32)

    inv = 1.0 / (ROPE_THETA ** (np.arange(0, HD, 2, dtype=np.float32) / HD))
    freqs = pos[:, None] * inv[None, :]                           # [S, 64]
    cos_r = np.ascontiguousarray(
        np.cos(freqs).astype(np.float32).reshape(ST, 128, 64).transpose(1, 0, 2)
    )
    sin_r = np.ascontiguousarray(
        np.sin(freqs).astype(np.float32).reshape(ST, 128, 64).transpose(1, 0, 2)
    )
    ht_r = np.ascontiguousarray(hid.T.reshape(KT, 128, S).transpose(1, 0, 2))

    in_maps = []
    for c in range(NCORES):
        kv = c // 2
        wqkv_c = np.concatenate(
            [
                Wq[:, 256 * c : 256 * (c + 1)],
                Wk[:, 128 * kv : 128 * (kv + 1)],
                Wv[:, 128 * kv : 128 * (kv + 1)],
            ],
            axis=1,
        )                                                          # [2048, 512]
        wqkv_r = np.ascontiguousarray(
            wqkv_c.reshape(KT, 128, 512).transpose(1, 0, 2)
        )
        wo_r = np.ascontiguousarray(
            Wo[256 * c : 256 * (c + 1), :].reshape(HPC, 128, HID).transpose(1, 0, 2)
        )
        in_maps.append(
            {
                "ht": ht_r,
                "wqkv": wqkv_r,
                "wo": wo_r,
                "cosr": cos_r,
                "sinr": sin_r,
                "identd": np.eye(128, dtype=np.float32),
                "trimaskd": np.triu(np.ones((128, 128), np.float32)),
                "onesd": np.ones((128, 128), np.float32),
            }
        )
    return in_maps


def run_spmd(in_maps, **kw):
    nc = get_nc()
    return bass_utils.run_bass_kernel_spmd(
        nc, in_maps, core_ids=list(range(NCORES)), **kw
    )


def kernel(hidden_states, position_ids, Wq, Wk, Wv, Wo):
    in_maps = prep_in_maps(hidden_states, position_ids, Wq, Wk, Wv, Wo)
    res = run_spmd(in_maps)
    total = res.results[0]["out"].astype(np.float32)
    for c in range(1, NCORES):
        total = total + res.results[c]["out"]
    return total[None]



# revision 2
# speedup vs baseline: 1.0746x; 1.0746x over previous
"""Llama GQA causal attention layer (the "topk" in the module name is a
mathematical identity) on 8 Trainium2 NeuronCores.

Sharding: tensor-parallel over heads. Each core owns 2 of the 16 q-heads and
the single kv-head they share, computes its slice of Q/K/V projections, RoPE,
causal flash attention (scores kept on-chip in [k, q] orientation), and a
row-slice o_proj producing a full-shape [S, HID] partial; the host sums the 8
partials (the o_proj row-parallel AllReduce done on host).

v2 changes vs baseline:
  - fp16 operands for every matmul (fp32 PSUM accumulate): enables fast
    weight loads (FWL), halves DMA + SBUF traffic, 2x DVE on 16-bit ops.
  - softmax denominator: instead of a ones-matmul per k-tile (1/3 of
    attention PE cycles), accumulate exp'd P tiles into a running fp32
    P_sum on the Vector engine and do ONE [1,512] matmul per (chunk, head).
  - fp16 output DMA (host sums partials in fp32).

Shapes hardcoded per problem spec:
  hidden_states [1, 4096, 2048] f32, position_ids [1, 4096] i32,
  Wq [2048, 2048], Wk/Wv [2048, 512], Wo [2048, 2048] f32.
"""

import math
import sys

import numpy as np

if "/opt/trn_rl_repo" not in sys.path:
    sys.path.insert(0, "/opt/trn_rl_repo")

import concourse.bass as bass
import concourse.mybir as mybir
import concourse.tile as tile
from concourse import bacc, bass_utils

B, S, HID = 1, 4096, 2048
NH, KVH, HD = 16, 4, 128
GROUPS = NH // KVH
NCORES = 8
HPC = NH // NCORES          # q heads per core = 2
ST = S // 128               # 32 s-tiles
KT = HID // 128             # 16 hid-tiles (contraction)
QCH = 512                   # q chunk width for attention
NQC = S // QCH
ROPE_THETA = 10000.0
ISQ = 1.0 / math.sqrt(HD)

F32 = mybir.dt.float32
F32R = mybir.dt.float32r
F16 = mybir.dt.float16


def build_body(tc, out, ht, wqkv, wo, cosr, sinr, identd, trimaskd, onesd):
    """Emit the per-core program, software-pipelined per 512-token q-chunk:
    for each chunk: [A] project+RoPE+transpose 4 s-tiles -> [B] causal flash
    attention for both heads -> [C] o_proj rows of this chunk. Emission order
    sets Tile priorities so PE stays dense across stage boundaries.

    DRAM layouts (host pre-arranged, partition dim first, fp16):
      ht    [128, KT, S]   ht[p, t, s]  = hidden[s, 128 t + p]
      wqkv  [128, KT, 512] wqkv[p,t,j]  = [Wq_c | Wk_c | Wv_c][128 t + p, j]
      wo    [128, HPC, HID] wo[p, j, n] = Wo[256 c + 128 j + p, n]
      cosr/sinr [128, ST, 64] cosr[p, st, f] = cos[128 st + p, f]
      out   [S, HID] fp16 partial output (sum over cores on host)
    """
    nc = tc.nc

    with (
        tc.tile_pool(name="const", bufs=1) as constp,
        tc.tile_pool(name="slabs", bufs=1) as slabs,
        tc.tile_pool(name="stream", bufs=3) as sp,
        tc.tile_pool(name="qtp", bufs=2) as qtp,
        tc.tile_pool(name="atp", bufs=2) as atp,
        tc.tile_pool(name="bsb", bufs=4) as bp,
        tc.tile_pool(name="csb", bufs=2) as cp,
        tc.tile_pool(name="ps512", bufs=4, space="PSUM") as ps512,
        tc.tile_pool(name="attps", bufs=2, space="PSUM") as attp,
        tc.tile_pool(name="denps", bufs=1, space="PSUM") as denp,
        tc.tile_pool(name="ptps", bufs=1, space="PSUM") as ptp,
    ):
        # Resident constants. wqkv split per hid-tile so compute starts early.
        wqkv_sb = constp.tile([128, KT, 512], F16)
        for t in range(KT):
            nc.sync.dma_start(out=wqkv_sb[:, t, :], in_=wqkv[:, t, :])
        cos_sb = constp.tile([128, ST, 64], F16)
        nc.sync.dma_start(out=cos_sb, in_=cosr)
        sin_sb = constp.tile([128, ST, 64], F16)
        nc.sync.dma_start(out=sin_sb, in_=sinr)
        wo_sb = constp.tile([128, HPC, HID], F16)
        nc.sync.dma_start(out=wo_sb, in_=wo)
        ident = constp.tile([128, 128], F16)
        nc.sync.dma_start(out=ident, in_=identd)
        # trimask[ki, qi] = 1.0 where qi >= ki else 0 (valid causal, k-major)
        trimask = constp.tile([128, 128], F16)
        nc.sync.dma_start(out=trimask, in_=trimaskd)
        ones_col = constp.tile([128, 1], F32R)
        nc.sync.dma_start(out=ones_col, in_=onesd[:, 0:1])

        kTc, vc = {}, {}
        for qc in range(NQC):
            # ---------------- A: project + RoPE + transpose ----------------
            qt = qtp.tile([128, HPC, QCH], F16, tag="qt")
            kt_c = slabs.tile([128, QCH], F16, tag=f"kT{qc}")
            v_c = slabs.tile([128, 4, 128], F16, tag=f"v{qc}")
            kTc[qc], vc[qc] = kt_c, v_c
            for si in range(4):
                st = 4 * qc + si
                s0 = st * 128
                sl = si * 128
                hs = sp.tile([128, KT, 128], F16, tag="hs")
                nc.sync.dma_start(out=hs, in_=ht[:, :, s0 : s0 + 128])
                pq = ps512.tile([128, 512], F32, tag="ps512")
                for t in range(KT):
                    nc.tensor.matmul(
                        pq,
                        lhsT=hs[:, t, :],
                        rhs=wqkv_sb[:, t, :],
                        start=(t == 0),
                        stop=(t == KT - 1),
                    )
                pqv = pq.rearrange("p (j d) -> p j d", d=128)
                c = cos_sb[:, st, :]
                s_ = sin_sb[:, st, :]
                rot = sp.tile([128, 3, 128], F16, tag="rot")
                t1 = sp.tile([128, 3, 64], F32, tag="t1")
                t2 = sp.tile([128, 3, 64], F32, tag="t2")
                for j in range(3):
                    a = pqv[:, j, 0:64]
                    b = pqv[:, j, 64:128]
                    nc.vector.tensor_mul(t1[:, j, :], a, c)
                    nc.vector.tensor_mul(t2[:, j, :], b, s_)
                    nc.vector.tensor_sub(rot[:, j, 0:64], t1[:, j, :], t2[:, j, :])
                    nc.vector.tensor_mul(t1[:, j, :], b, c)
                    nc.vector.tensor_mul(t2[:, j, :], a, s_)
                    nc.vector.tensor_add(rot[:, j, 64:128], t1[:, j, :], t2[:, j, :])
                nc.scalar.copy(v_c[:, si, :], pqv[:, 3, :])
                for j, dest, eng in (
                    (0, qt[:, 0, sl : sl + 128], "act"),
                    (1, qt[:, 1, sl : sl + 128], "dve"),
                    (2, kt_c[:, sl : sl + 128], "dve"),
                ):
                    pt = ptp.tile([128, 128], F16, tag="pt")
                    nc.tensor.transpose(pt, rot[:, j, :], ident)
                    if eng == "act":
                        nc.scalar.copy(dest, pt)
                    else:
                        nc.vector.tensor_copy(dest, pt)

            # ---------------- B: causal flash attention --------------------
            q0 = qc * QCH
            at_c = atp.tile([128, HPC, QCH], F16, tag="att")
            nkt = 4 * qc + 4
            for h in range(HPC):
                att_ps = attp.tile([128, QCH], F32, tag="attps")
                psum_sb = bp.tile([128, QCH], F32R, tag="psum")
                for kt in range(nkt):
                    k0 = kt * 128
                    kch, kx = kt // 4, kt % 4
                    off = max(0, k0 - q0)
                    ps = ps512.tile([128, QCH], F32, tag="ps512")
                    nc.tensor.matmul(
                        ps[:, off:],
                        lhsT=kTc[kch][:, 128 * kx : 128 * kx + 128],
                        rhs=qt[:, h, off:QCH],
                        start=True,
                        stop=True,
                    )
                    pt_sb = bp.tile([128, QCH], F16, tag="pT")
                    nc.scalar.activation(
                        pt_sb[:, off:],
                        ps[:, off:],
                        mybir.ActivationFunctionType.Exp,
                        scale=ISQ,
                    )
                    if kt >= 4 * qc:  # diagonal block: zero strictly-future q
                        nc.vector.tensor_mul(
                            pt_sb[:, off : off + 128],
                            pt_sb[:, off : off + 128],
                            trimask,
                        )
                    if kt == 0:
                        nc.vector.tensor_copy(psum_sb, pt_sb)
                    else:
                        nc.vector.tensor_add(
                            psum_sb[:, off:], psum_sb[:, off:], pt_sb[:, off:]
                        )
                    nc.tensor.matmul(
                        att_ps[:, off:],
                        lhsT=vc[kch][:, kx, :],
                        rhs=pt_sb[:, off:],
                        start=(kt == 0),
                        stop=(kt == nkt - 1),
                    )
                den_ps = denp.tile([1, QCH], F32, tag="denps")
                nc.tensor.matmul(
                    den_ps, lhsT=ones_col, rhs=psum_sb, start=True, stop=True
                )
                den_sb = bp.tile([1, QCH], F32, tag="den")
                nc.vector.tensor_copy(den_sb, den_ps)
                rden = bp.tile([1, QCH], F32, tag="rden")
                nc.vector.reciprocal_approx_fast(rden, den_sb)
                rdb = bp.tile([128, QCH], F32, tag="rdb")
                nc.gpsimd.partition_broadcast(rdb, rden)
                nc.vector.tensor_mul(at_c[:, h, :], att_ps, rdb)

            # ---------------- C: o_proj rows of this chunk -----------------
            for si in range(4):
                st = 4 * qc + si
                s0 = st * 128
                sl = si * 128
                osb = cp.tile([128, HID], F16, tag="osb")
                for nch in range(HID // 512):
                    n0 = nch * 512
                    po = ps512.tile([128, 512], F32, tag="ps512")
                    for j in range(HPC):
                        nc.tensor.matmul(
                            po,
                            lhsT=at_c[:, j, sl : sl + 128],
                            rhs=wo_sb[:, j, n0 : n0 + 512],
                            start=(j == 0),
                            stop=(j == HPC - 1),
                        )
                    if nch % 2 == 0:
                        nc.scalar.copy(osb[:, n0 : n0 + 512], po)
                    else:
                        nc.vector.tensor_copy(osb[:, n0 : n0 + 512], po)
                nc.sync.dma_start(out=out[s0 : s0 + 128, :], in_=osb)


_NC_CACHE = {}


def get_nc():
    key = "nc"
    if key not in _NC_CACHE:
        nc = bacc.Bacc(
            "TRN2",
            debug=False,
            enable_asserts=False,
            target_bir_lowering=False,
        )
        ht = nc.dram_tensor("ht", [128, KT, S], F16, kind="ExternalInput").ap()
        wqkv = nc.dram_tensor("wqkv", [128, KT, 512], F16, kind="ExternalInput").ap()
        wo = nc.dram_tensor("wo", [128, HPC, HID], F16, kind="ExternalInput").ap()
        cosr = nc.dram_tensor("cosr", [128, ST, 64], F16, kind="ExternalInput").ap()
        sinr = nc.dram_tensor("sinr", [128, ST, 64], F16, kind="ExternalInput").ap()
        identd = nc.dram_tensor("identd", [128, 128], F16, kind="ExternalInput").ap()
        trimaskd = nc.dram_tensor("trimaskd", [128, 128], F16, kind="ExternalInput").ap()
        onesd = nc.dram_tensor("onesd", [128, 1], F32R, kind="ExternalInput").ap()
        out = nc.dram_tensor("out", [S, HID], F16, kind="ExternalOutput").ap()
        with tile.TileContext(nc) as tc:
            build_body(tc, out, ht, wqkv, wo, cosr, sinr, identd, trimaskd, onesd)
        nc.compile()
        _NC_CACHE[key] = nc
    return _NC_CACHE[key]


def prep_in_maps(hidden_states, position_ids, Wq, Wk, Wv, Wo):
    hid = np.asarray(hidden_states, dtype=np.float32)[0]          # [S, HID]
    pos = np.asarray(position_ids)[0].astype(np.float32)          # [S]
    Wq = np.asarray(Wq, dtype=np.float32)
    Wk = np.asarray(Wk, dtype=np.float32)
    Wv = np.asarray(Wv, dtype=np.float32)
    Wo = np.asarray(Wo, dtype=np.float32)

    inv = 1.0 / (ROPE_THETA ** (np.arange(0, HD, 2, dtype=np.float32) / HD))
    freqs = pos[:, None] * inv[None, :]                           # [S, 64]
    cos_r = np.ascontiguousarray(
        np.cos(freqs).reshape(ST, 128, 64).transpose(1, 0, 2)
    ).astype(np.float16)
    sin_r = np.ascontiguousarray(
        np.sin(freqs).reshape(ST, 128, 64).transpose(1, 0, 2)
    ).astype(np.float16)
    ht_r = np.ascontiguousarray(
        hid.T.reshape(KT, 128, S).transpose(1, 0, 2)
    ).astype(np.float16)

    in_maps = []
    for c in range(NCORES):
        kv = c // 2
        wqkv_c = np.concatenate(
            [
                Wq[:, 256 * c : 256 * (c + 1)],
                Wk[:, 128 * kv : 128 * (kv + 1)],
                Wv[:, 128 * kv : 128 * (kv + 1)],
            ],
            axis=1,
        )                                                          # [2048, 512]
        wqkv_r = np.ascontiguousarray(
            wqkv_c.reshape(KT, 128, 512).transpose(1, 0, 2)
        ).astype(np.float16)
        wo_r = np.ascontiguousarray(
            Wo[256 * c : 256 * (c + 1), :].reshape(HPC, 128, HID).transpose(1, 0, 2)
        ).astype(np.float16)
        in_maps.append(
            {
                "ht": ht_r,
                "wqkv": wqkv_r,
                "wo": wo_r,
                "cosr": cos_r,
                "sinr": sin_r,
                "identd": np.eye(128, dtype=np.float16),
                "trimaskd": np.triu(np.ones((128, 128), np.float16)),
                "onesd": np.ones((128, 1), np.float32),
            }
        )
    return in_maps


def run_spmd(in_maps, **kw):
    nc = get_nc()
    return bass_utils.run_bass_kernel_spmd(
        nc, in_maps, core_ids=list(range(NCORES)), **kw
    )


def kernel(hidden_states, position_ids, Wq, Wk, Wv, Wo):
    in_maps = prep_in_maps(hidden_states, position_ids, Wq, Wk, Wv, Wo)
    res = run_spmd(in_maps)
    total = res.results[0]["out"].astype(np.float32)
    for c in range(1, NCORES):
        total = total + res.results[c]["out"].astype(np.float32)
    return total[None]


# revision 3
# speedup vs baseline: 1.3471x; 1.2536x over previous
"""Llama GQA causal attention layer (the "topk" in the module name is a
mathematical identity) on 8 Trainium2 NeuronCores.

Sharding: tensor-parallel over heads. Each core owns 2 of the 16 q-heads and
the single kv-head they share, computes its slice of Q/K/V projections, RoPE,
causal flash attention (scores kept on-chip in [k, q] orientation), and a
row-slice o_proj producing a full-shape [S, HID] partial; the host sums the 8
partials (the o_proj row-parallel AllReduce done on host).

v3 changes vs v2:
  - B-stage software pipeline: scores(kt+1) emitted before AV(kt) so PE
    never stalls on the exp->mask chain.
  - P_sum pair-tree: off-diagonal k-tile groups of 4 summed in fp16 (2x DVE
    mode) then one fp32 accumulate; diagonal tiles added fp32 on partial
    widths. Softmax denominator = ONE ones-matmul per (chunk, head).
  - RoPE batched into [128,3,64] strided DVE ops against host-tripled
    cos/sin tables (576 -> 192 DVE instructions).
  - Chunk pipeline reorder: den/normalize/o_proj of chunk qc-1 are emitted
    AFTER A(qc), so projection matmuls hide the den-path latency.
  - Startup: wqkv DMA first on the sync queue (chunk-0 hs right behind);
    all other constants on the scalar engine's DMA queue.
"""

import math
import sys

import numpy as np

if "/opt/trn_rl_repo" not in sys.path:
    sys.path.insert(0, "/opt/trn_rl_repo")

import concourse.bass as bass
import concourse.mybir as mybir
import concourse.tile as tile
from concourse import bacc, bass_utils

B, S, HID = 1, 4096, 2048
NH, KVH, HD = 16, 4, 128
GROUPS = NH // KVH
NCORES = 8
HPC = NH // NCORES          # q heads per core = 2
ST = S // 128               # 32 s-tiles
KT = HID // 128             # 16 hid-tiles (contraction)
QCH = 512                   # q chunk width for attention
NQC = S // QCH
ROPE_THETA = 10000.0
ISQ = 1.0 / math.sqrt(HD)

F32 = mybir.dt.float32
F32R = mybir.dt.float32r
F16 = mybir.dt.float16


def build_body(tc, out, ht, wqkv, wo, cos3d, sin3d, identd, trimaskd, onesd):
    """DRAM layouts (host pre-arranged, partition dim first, fp16):
      ht    [128, KT, S]    ht[p, t, s] = hidden[s, 128 t + p]
      wqkv  [128, KT, 512]  wqkv[p,t,j] = [Wq_c | Wk_c | Wv_c][128 t + p, j]
      wo    [128, HPC, HID] wo[p, j, n] = Wo[256 c + 128 j + p, n]
      cos3d/sin3d [128, ST, 3, 64]  cos tripled along a j axis for batched rope
      out   [S, HID] fp16 partial output (sum over cores on host)
    """
    nc = tc.nc

    with (
        tc.tile_pool(name="const", bufs=1) as constp,
        tc.tile_pool(name="slabs", bufs=1) as slabs,
        tc.tile_pool(name="stream", bufs=3) as sp,
        tc.tile_pool(name="qtp", bufs=2) as qtp,
        tc.tile_pool(name="atp", bufs=2) as atp,
        tc.tile_pool(name="ptb", bufs=8) as ptb,
        tc.tile_pool(name="grp", bufs=2) as grp,
        tc.tile_pool(name="bsb", bufs=3) as bp,
        tc.tile_pool(name="csb", bufs=2) as cp,
        tc.tile_pool(name="ps512", bufs=4, space="PSUM") as ps512,
        tc.tile_pool(name="attps", bufs=2, space="PSUM") as attp,
        tc.tile_pool(name="denps", bufs=1, space="PSUM") as denp,
        tc.tile_pool(name="ptps", bufs=1, space="PSUM") as ptp,
    ):
        # wqkv first on the sync queue so chunk-0 hs tiles land right behind.
        wqkv_sb = constp.tile([128, KT, 512], F16)
        for t in range(KT):
            nc.sync.dma_start(out=wqkv_sb[:, t, :], in_=wqkv[:, t, :])
        # Remaining constants go on the scalar engine's DMA queue.
        cos_sb = constp.tile([128, ST, 3, 64], F16)
        nc.scalar.dma_start(out=cos_sb, in_=cos3d)
        sin_sb = constp.tile([128, ST, 3, 64], F16)
        nc.scalar.dma_start(out=sin_sb, in_=sin3d)
        ident = constp.tile([128, 128], F16)
        nc.scalar.dma_start(out=ident, in_=identd)
        # trimask[ki, qi] = 1.0 where qi >= ki else 0 (valid causal, k-major)
        trimask = constp.tile([128, 128], F16)
        nc.scalar.dma_start(out=trimask, in_=trimaskd)
        ones_col = constp.tile([128, 1], F32R)
        nc.scalar.dma_start(out=ones_col, in_=onesd[:, 0:1])
        wo_sb = constp.tile([128, HPC, HID], F16)
        nc.scalar.dma_start(out=wo_sb, in_=wo)

        kTc, vc = {}, {}
        state = {}  # qc -> (qt, att_ps[h], psum32[h])

        def stage_A(qc):
            qt = qtp.tile([128, HPC, QCH], F16, tag="qt")
            kt_c = slabs.tile([128, QCH], F16, tag=f"kT{qc}")
            v_c = slabs.tile([128, 4, 128], F16, tag=f"v{qc}")
            kTc[qc], vc[qc] = kt_c, v_c
            for si in range(4):
                st = 4 * qc + si
                s0 = st * 128
                sl = si * 128
                hs = sp.tile([128, KT, 128], F16, tag="hs")
                nc.sync.dma_start(out=hs, in_=ht[:, :, s0 : s0 + 128])
                pq = ps512.tile([128, 512], F32, tag="ps512")
                for t in range(KT):
                    nc.tensor.matmul(
                        pq,
                        lhsT=hs[:, t, :],
                        rhs=wqkv_sb[:, t, :],
                        start=(t == 0),
                        stop=(t == KT - 1),
                    )
                pqv = pq.rearrange("p (j d) -> p j d", d=128)
                # batched rope over j=0..2 (q0, q1, k) as [128,3,64] strided ops
                a = pqv[:, 0:3, 0:64]
                b = pqv[:, 0:3, 64:128]
                c3 = cos_sb[:, st, :, :]
                s3 = sin_sb[:, st, :, :]
                rot = sp.tile([128, 3, 128], F16, tag="rot")
                t1 = sp.tile([128, 3, 64], F32, tag="t1")
                t2 = sp.tile([128, 3, 64], F32, tag="t2")
                nc.vector.tensor_mul(t1, a, c3)
                nc.vector.tensor_mul(t2, b, s3)
                nc.vector.tensor_sub(rot[:, :, 0:64], t1, t2)
                nc.vector.tensor_mul(t1, b, c3)
                nc.vector.tensor_mul(t2, a, s3)
                nc.vector.tensor_add(rot[:, :, 64:128], t1, t2)
                nc.scalar.copy(v_c[:, si, :], pqv[:, 3, :])
                # 3 transposes into one PSUM tile, then 2 evac copies
                pt3 = ptp.tile([128, 3, 128], F16, tag="pt3")
                for j in range(3):
                    nc.tensor.transpose(pt3[:, j, :], rot[:, j, :], ident)
                nc.scalar.copy(qt[:, :, sl : sl + 128], pt3[:, 0:2, :])
                nc.vector.tensor_copy(kt_c[:, sl : sl + 128], pt3[:, 2, :])
            return qt

        def stage_B(qc, qt):
            q0 = qc * QCH
            nkt = 4 * qc + 4
            atts, psums = [], []
            for h in range(HPC):
                att_ps = attp.tile([128, QCH], F32, tag="attps")
                psum32 = bp.tile([128, QCH], F32R, tag="psum32")
                atts.append(att_ps)
                psums.append(psum32)
                prev = None          # (kt, off, pt)
                group = []           # [(kt, off, pt)] pending P_sum tiles
                initialized = False

                def flush_group():
                    nonlocal initialized, group
                    if not group:
                        return
                    if group[0][1] == 0 and group[-1][1] == 0 and len(group) == 4:
                        # full-width off-diagonal group: fp16 pair tree
                        pa = grp.tile([128, QCH], F16, tag="pa")
                        pb = grp.tile([128, QCH], F16, tag="pb")
                        pg = grp.tile([128, QCH], F16, tag="pg")
                        nc.vector.tensor_add(pa, group[0][2], group[1][2])
                        nc.vector.tensor_add(pb, group[2][2], group[3][2])
                        nc.vector.tensor_add(pg, pa, pb)
                        if not initialized:
                            nc.vector.tensor_copy(psum32, pg)
                            initialized = True
                        else:
                            nc.vector.tensor_add(psum32, psum32, pg)
                    else:
                        # diagonal group: partial-width fp32 adds
                        for kt_, off_, pt_ in group:
                            if not initialized:
                                nc.vector.tensor_copy(psum32, pt_)
                                initialized = True
                            else:
                                nc.vector.tensor_add(
                                    psum32[:, off_:], psum32[:, off_:], pt_[:, off_:]
                                )
                    group = []

                def emit_av(item, is_last):
                    kt_, off_, pt_ = item
                    nc.tensor.matmul(
                        att_ps[:, off_:],
                        lhsT=vc[kt_ // 4][:, kt_ % 4, :],
                        rhs=pt_[:, off_:],
                        start=(kt_ == 0),
                        stop=is_last,
                    )

                for kt in range(nkt):
                    k0 = kt * 128
                    off = max(0, k0 - q0)
                    ps = ps512.tile([128, QCH], F32, tag="ps512")
                    nc.tensor.matmul(
                        ps[:, off:],
                        lhsT=kTc[kt // 4][:, 128 * (kt % 4) : 128 * (kt % 4) + 128],
                        rhs=qt[:, h, off:QCH],
                        start=True,
                        stop=True,
                    )
                    pt = ptb.tile([128, QCH], F16, tag="pT")
                    nc.scalar.activation(
                        pt[:, off:],
                        ps[:, off:],
                        mybir.ActivationFunctionType.Exp,
                        scale=ISQ,
                    )
                    if kt >= 4 * qc:  # diagonal block: zero strictly-future q
                        nc.vector.tensor_mul(
                            pt[:, off : off + 128], pt[:, off : off + 128], trimask
                        )
                    if prev is not None:
                        emit_av(prev, False)
                    group.append((kt, off, pt))
                    if len(group) == 4:
                        flush_group()
                    prev = (kt, off, pt)
                emit_av(prev, True)
                flush_group()
            state[qc] = (atts, psums)

        def stage_den_C(qc):
            atts, psums = state.pop(qc)
            at_c = atp.tile([128, HPC, QCH], F16, tag="att")
            for h in range(HPC):
                den_ps = denp.tile([1, QCH], F32, tag="denps")
                nc.tensor.matmul(
                    den_ps, lhsT=ones_col, rhs=psums[h], start=True, stop=True
                )
                den_sb = bp.tile([1, QCH], F32, tag="den")
                nc.vector.tensor_copy(den_sb, den_ps)
                rden = bp.tile([1, QCH], F32, tag="rden")
                nc.vector.reciprocal_approx_fast(rden, den_sb)
                rdb = bp.tile([128, QCH], F32, tag="rdb")
                nc.gpsimd.partition_broadcast(rdb, rden)
                nc.vector.tensor_mul(at_c[:, h, :], atts[h], rdb)
            for si in range(4):
                s0 = (4 * qc + si) * 128
                sl = si * 128
                osb = cp.tile([128, HID], F16, tag="osb")
                for nch in range(HID // 512):
                    n0 = nch * 512
                    po = ps512.tile([128, 512], F32, tag="ps512")
                    for j in range(HPC):
                        nc.tensor.matmul(
                            po,
                            lhsT=at_c[:, j, sl : sl + 128],
                            rhs=wo_sb[:, j, n0 : n0 + 512],
                            start=(j == 0),
                            stop=(j == HPC - 1),
                        )
                    if nch % 2 == 0:
                        nc.scalar.copy(osb[:, n0 : n0 + 512], po)
                    else:
                        nc.vector.tensor_copy(osb[:, n0 : n0 + 512], po)
                nc.sync.dma_start(out=out[s0 : s0 + 128, :], in_=osb)

        # Pipeline: A0 B0 | A1 dnC0 B1 | A2 dnC1 B2 | ... | dnC7
        for qc in range(NQC):
            qt = stage_A(qc)
            if qc > 0:
                stage_den_C(qc - 1)
            stage_B(qc, qt)
        stage_den_C(NQC - 1)


_NC_CACHE = {}


def get_nc():
    key = "nc"
    if key not in _NC_CACHE:
        nc = bacc.Bacc(
            "TRN2",
            debug=False,
            enable_asserts=False,
            target_bir_lowering=False,
        )
        ht = nc.dram_tensor("ht", [128, KT, S], F16, kind="ExternalInput").ap()
        wqkv = nc.dram_tensor("wqkv", [128, KT, 512], F16, kind="ExternalInput").ap()
        wo = nc.dram_tensor("wo", [128, HPC, HID], F16, kind="ExternalInput").ap()
        cos3d = nc.dram_tensor("cos3d", [128, ST, 3, 64], F16, kind="ExternalInput").ap()
        sin3d = nc.dram_tensor("sin3d", [128, ST, 3, 64], F16, kind="ExternalInput").ap()
        identd = nc.dram_tensor("identd", [128, 128], F16, kind="ExternalInput").ap()
        trimaskd = nc.dram_tensor("trimaskd", [128, 128], F16, kind="ExternalInput").ap()
        onesd = nc.dram_tensor("onesd", [128, 1], F32R, kind="ExternalInput").ap()
        out = nc.dram_tensor("out", [S, HID], F16, kind="ExternalOutput").ap()
        with tile.TileContext(nc) as tc:
            build_body(tc, out, ht, wqkv, wo, cos3d, sin3d, identd, trimaskd, onesd)
        nc.compile()
        _NC_CACHE[key] = nc
    return _NC_CACHE[key]


def prep_in_maps(hidden_states, position_ids, Wq, Wk, Wv, Wo):
    hid = np.asarray(hidden_states, dtype=np.float32)[0]          # [S, HID]
    pos = np.asarray(position_ids)[0].astype(np.float32)          # [S]
    Wq = np.asarray(Wq, dtype=np.float32)
    Wk = np.asarray(Wk, dtype=np.float32)
    Wv = np.asarray(Wv, dtype=np.float32)
    Wo = np.asarray(Wo, dtype=np.float32)

    inv = 1.0 / (ROPE_THETA ** (np.arange(0, HD, 2, dtype=np.float32) / HD))
    freqs = pos[:, None] * inv[None, :]                           # [S, 64]
    cos_r = np.cos(freqs).reshape(ST, 128, 64).transpose(1, 0, 2)  # [128,ST,64]
    sin_r = np.sin(freqs).reshape(ST, 128, 64).transpose(1, 0, 2)
    cos3 = np.ascontiguousarray(
        np.repeat(cos_r[:, :, None, :], 3, axis=2)
    ).astype(np.float16)                                           # [128,ST,3,64]
    sin3 = np.ascontiguousarray(
        np.repeat(sin_r[:, :, None, :], 3, axis=2)
    ).astype(np.float16)
    ht_r = np.ascontiguousarray(
        hid.T.reshape(KT, 128, S).transpose(1, 0, 2)
    ).astype(np.float16)

    in_maps = []
    for c in range(NCORES):
        kv = c // 2
        wqkv_c = np.concatenate(
            [
                Wq[:, 256 * c : 256 * (c + 1)],
                Wk[:, 128 * kv : 128 * (kv + 1)],
                Wv[:, 128 * kv : 128 * (kv + 1)],
            ],
            axis=1,
        )                                                          # [2048, 512]
        wqkv_r = np.ascontiguousarray(
            wqkv_c.reshape(KT, 128, 512).transpose(1, 0, 2)
        ).astype(np.float16)
        wo_r = np.ascontiguousarray(
            Wo[256 * c : 256 * (c + 1), :].reshape(HPC, 128, HID).transpose(1, 0, 2)
        ).astype(np.float16)
        in_maps.append(
            {
                "ht": ht_r,
                "wqkv": wqkv_r,
                "wo": wo_r,
                "cos3d": cos3,
                "sin3d": sin3,
                "identd": np.eye(128, dtype=np.float16),
                "trimaskd": np.triu(np.ones((128, 128), np.float16)),
                "onesd": np.ones((128, 1), np.float32),
            }
        )
    return in_maps


def run_spmd(in_maps, **kw):
    nc = get_nc()
    return bass_utils.run_bass_kernel_spmd(
        nc, in_maps, core_ids=list(range(NCORES)), **kw
    )


def kernel(hidden_states, position_ids, Wq, Wk, Wv, Wo):
    in_maps = prep_in_maps(hidden_states, position_ids, Wq, Wk, Wv, Wo)
    res = run_spmd(in_maps)
    total = res.results[0]["out"].astype(np.float32)
    for c in range(1, NCORES):
        total = total + res.results[c]["out"].astype(np.float32)
    return total[None]
